# revision 3
# baseline (speedup 1.0000x reference)
"""KAN projection kernel for 8x Trainium2 NeuronCores.

Math (reference): out = silu(x) @ scale_base + einsum('ndg,dog->no', B(x), coef*scale_sp)
where B are cubic B-spline bases on a uniform extended grid over [-1,1],
GRID=5, K=3 -> G+K = 8 basis functions per input dim.

Key reformulation: for a *uniform* grid every dense basis value is
B[n,d,s] = sum_{c} 1(cell==c) * b_{s-c}(u),  cell=floor(t), u=frac(t),
t=(x - knot0)/h, with the 4 standard uniform cubic blending polynomials
b_j(u). So the contraction 'ndg,dog->no' splits into 8 per-plane matmuls
over d only (plane s uses weight slice coef[:,:,s]), plus a 9th plane
silu(x) @ scale_base. All planes live in (d, token) layout on-chip: no
cross-partition scatter is ever needed. The 1/6 normalization of the
blending polys is folded into the coefficients on the host.

Sharding: data-parallel over the 8192 tokens (1024 tokens/core). Host
pre-transposes x to (d, tok) and pre-packs weights to (72, 128, O) =
(d-chunk x plane, d_local, o). Matmul dtype float32r (TF32-class, full
PE rate at free-dim>=256). Output computed in natural (tok, o) layout.
"""

import sys

sys.path.insert(0, '/opt/trn_rl_repo')

import numpy as np

import concourse.bass as bass  # noqa: F401  (bass must import before mybir use)
import concourse.mybir as mybir
from concourse import bacc
from concourse.tile import TileContext
from concourse.bass_utils import run_bass_kernel_spmd

F32 = mybir.dt.float32
F32R = mybir.dt.float32r
ALU = mybir.AluOpType
ACTF = mybir.ActivationFunctionType

D = 1024            # input dim
O = 2048            # output dim
NTOK = 8192         # flattened tokens
NCORES = 8
TPC = NTOK // NCORES  # tokens per core = 1024
TS = 512            # token supertile
NDC = D // 128      # d chunks = 8
NPL = 9             # 8 spline planes + 1 silu/base plane
NKI = NDC * NPL     # 72 contraction steps of K=128
OC = 512            # output-dim chunk per psum tile
MAGIC = 8388608.0   # 2^23 float32 round-to-nearest trick

_CACHE = {}
TRACE = False
LAST_EXEC_NS = None


def _build(scale: float, bias: float, reps: int = 1):
    """Per-core kernel: xt (D, TPC) f32, w (NKI, 128, O) f32r -> out (TPC, O) f32.

    t = x*scale + bias maps x into knot-index space [0, GRID).
    reps>1 repeats the whole body (for delta-iterations HW timing)."""
    nc = bacc.Bacc(None, target_bir_lowering=False, debug=False)
    with TileContext(nc) as tc:
        with tc.tile_pool(name="dram", bufs=1, space="DRAM") as dram:
            xt = dram.tile([D, TPC], F32, kind="ExternalInput", tag="xt")
            w = dram.tile([NKI, 128, O], F32R, kind="ExternalInput", tag="w")
            out = dram.tile([TPC, O], F32, kind="ExternalOutput", tag="out")
            with (
                tc.tile_pool(name="bp", bufs=NKI) as bpp,
                tc.tile_pool(name="xp", bufs=2) as xpp,
                tc.tile_pool(name="tmp", bufs=2) as tmp,
                tc.tile_pool(name="wp", bufs=3) as wpp,
                tc.tile_pool(name="ev", bufs=2) as evp,
                tc.tile_pool(name="ps", bufs=8, space="PSUM") as psp,
            ):
                for ts_i in [t for _ in range(reps) for t in range(TPC // TS)]:
                    tok0 = ts_i * TS
                    planes = [None] * NKI
                    for dc in range(NDC):
                        xtile = xpp.tile([128, TS], F32, tag="x")
                        nc.sync.dma_start(
                            xtile[:], xt[dc * 128:(dc + 1) * 128, tok0:tok0 + TS])
                        # silu plane (ki = dc*NPL + 8)
                        sil = bpp.tile([128, TS], F32R, tag="bp")
                        nc.scalar.activation(sil[:], xtile[:], ACTF.Silu)
                        planes[dc * NPL + 8] = sil
                        for pl in range(8):
                            planes[dc * NPL + pl] = bpp.tile([128, TS], F32R, tag="bp", name=f"bp{dc}_{pl}")
                        # basis computation in two halves to bound temp SBUF
                        for h in range(2):
                            hs = slice(h * (TS // 2), (h + 1) * (TS // 2))
                            xh = xtile[:, hs]
                            t = tmp.tile([128, TS // 2], F32, tag="t")
                            nc.scalar.activation(t[:], xh, ACTF.Copy,
                                                 bias=bias, scale=scale)
                            r = tmp.tile([128, TS // 2], F32, tag="r")
                            nc.gpsimd.tensor_scalar(r[:], t[:], MAGIC, MAGIC,
                                                    ALU.add, ALU.subtract)
                            u = tmp.tile([128, TS // 2], F32, tag="u")
                            nc.vector.tensor_tensor(u[:], t[:], r[:], ALU.subtract)
                            m = tmp.tile([128, TS // 2], F32, tag="m")
                            nc.gpsimd.tensor_scalar(m[:], u[:], 0.0, None, ALU.is_lt)
                            nc.vector.tensor_tensor(u[:], u[:], m[:], ALU.add)
                            cell = tmp.tile([128, TS // 2], F32, tag="cell")
                            nc.gpsimd.tensor_tensor(cell[:], r[:], m[:], ALU.subtract)
                            oh = []
                            for c in range(5):
                                ohc = tmp.tile([128, TS // 2], F32, tag=f"oh{c}")
                                nc.gpsimd.tensor_scalar(ohc[:], cell[:], float(c),
                                                        None, ALU.is_equal)
                                oh.append(ohc)
                            u2 = tmp.tile([128, TS // 2], F32, tag="u2")
                            nc.vector.tensor_tensor(u2[:], u[:], u[:], ALU.mult)
                            u3 = tmp.tile([128, TS // 2], F32, tag="u3")
                            nc.vector.tensor_tensor(u3[:], u2[:], u[:], ALU.mult)
                            en = tmp.tile([128, TS // 2], F32, tag="en")
                            nc.scalar.activation(en[:], u[:], ACTF.Copy,
                                                 bias=1.0, scale=-1.0)
                            en2 = tmp.tile([128, TS // 2], F32, tag="en2")
                            nc.vector.tensor_tensor(en2[:], en[:], en[:], ALU.mult)
                            # 6*B polys: b0=(1-u)^3, b1=3u^3-6u^2+4,
                            # b2=-3u^3+3u^2+3u+1, b3=u^3
                            b0 = tmp.tile([128, TS // 2], F32, tag="b0")
                            nc.vector.tensor_tensor(b0[:], en2[:], en[:], ALU.mult)
                            a1 = tmp.tile([128, TS // 2], F32, tag="a1")
                            nc.gpsimd.tensor_scalar(a1[:], u[:], 3.0, -6.0,
                                                    ALU.mult, ALU.add)
                            b1 = tmp.tile([128, TS // 2], F32, tag="b1")
                            nc.vector.tensor_tensor(b1[:], a1[:], u2[:], ALU.mult)
                            nc.gpsimd.tensor_scalar(b1[:], b1[:], 4.0, None, ALU.add)
                            p2 = tmp.tile([128, TS // 2], F32, tag="p2")
                            nc.vector.scalar_tensor_tensor(p2[:], u3[:], -1.0, u2[:],
                                                           ALU.mult, ALU.add)
                            q2 = tmp.tile([128, TS // 2], F32, tag="q2")
                            nc.scalar.activation(q2[:], u[:], ACTF.Copy,
                                                 bias=1.0, scale=3.0)
                            b2 = tmp.tile([128, TS // 2], F32, tag="b2")
                            nc.vector.scalar_tensor_tensor(b2[:], p2[:], 3.0, q2[:],
                                                           ALU.mult, ALU.add)
                            bs = [b0, b1, b2, u3]
                            # plane s = sum_c oh[c] * b_{s-c}
                            for s in range(8):
                                dst = planes[dc * NPL + s][:, hs]
                                cs = [c for c in range(max(0, s - 3), min(4, s) + 1)]
                                first = True
                                for c in cs:
                                    bj = bs[s - c]
                                    if first:
                                        nc.vector.tensor_tensor(
                                            dst, bj[:], oh[c][:], ALU.mult)
                                        first = False
                                    else:
                                        pr = tmp.tile([128, TS // 2], F32, tag="pr")
                                        nc.vector.tensor_tensor(
                                            pr[:], bj[:], oh[c][:], ALU.mult)
                                        nc.gpsimd.tensor_tensor(
                                            dst, dst, pr[:], ALU.add)
                    # matmul sweep: out[tok0:tok0+TS, :] += planes.T @ w
                    for oc_i in range(O // OC):
                        ps = [psp.tile([128, OC], F32, name=f"ps{tt}", tag="ps") for tt in range(TS // 128)]
                        for ki in range(NKI):
                            wt = wpp.tile([128, OC], F32R, tag="w")
                            nc.sync.dma_start(
                                wt[:], w[ki, :, oc_i * OC:(oc_i + 1) * OC])
                            for tt in range(TS // 128):
                                nc.tensor.matmul(
                                    ps[tt][:],
                                    planes[ki][:, tt * 128:(tt + 1) * 128],
                                    wt[:],
                                    start=(ki == 0), stop=(ki == NKI - 1))
                        for tt in range(TS // 128):
                            ev = evp.tile([128, OC], F32, tag="ev")
                            nc.scalar.copy(ev[:], ps[tt][:])
                            nc.sync.dma_start(
                                out[tok0 + tt * 128:tok0 + (tt + 1) * 128,
                                    oc_i * OC:(oc_i + 1) * OC],
                                ev[:])
    nc.compile()
    return nc, xt.name, w.name, out.name


def kernel(x, grid, coef, scale_base, scale_sp):
    assert x.shape == (4, 2048, D) and x.dtype == np.float32
    xf = np.ascontiguousarray(x.reshape(NTOK, D))
    # uniform grid: t = (x - knot0)/h with knot0 = grid[0, K=3]
    h = float(grid[0, 1] - grid[0, 0])
    scale = 1.0 / h
    bias = -float(grid[0, 3]) / h
    key = (round(scale, 9), round(bias, 9))
    if key not in _CACHE:
        _CACHE[key] = _build(scale, bias)
    nc, xt_name, w_name, out_name = _CACHE[key]

    # pack weights: plane g<8 -> coef[:,:,g]*scale_sp/6 ; plane 8 -> scale_base
    coefs = (coef * scale_sp[:, :, None] * (1.0 / 6.0)).astype(np.float32)
    W = np.empty((NKI, 128, O), np.float32)
    for dc in range(NDC):
        dsl = slice(dc * 128, (dc + 1) * 128)
        for pl in range(8):
            W[dc * NPL + pl] = coefs[dsl, :, pl]
        W[dc * NPL + 8] = scale_base[dsl, :]

    xT = np.ascontiguousarray(xf.T)  # (D, NTOK)
    in_maps = []
    for c in range(NCORES):
        in_maps.append({
            xt_name: np.ascontiguousarray(xT[:, c * TPC:(c + 1) * TPC]),
            w_name: W,
        })
    res = run_bass_kernel_spmd(nc, in_maps, core_ids=list(range(NCORES)),
                               trace=TRACE)
    global LAST_EXEC_NS
    LAST_EXEC_NS = res.exec_time_ns
    out = np.concatenate([res.results[c][out_name] for c in range(NCORES)], axis=0)
    return out.reshape(4, 2048, O)



# revision 6
# speedup vs baseline: 1.0074x; 1.0074x over previous
"""KAN projection kernel for 8x Trainium2 NeuronCores — v2.

Math: out = silu(x) @ scale_base + einsum('ndg,dog->no', B(x), coef*scale_sp)
with cubic B-splines (GRID=5, K=3 -> 8 basis functions) on a uniform grid
over [-1,1].

Reformulation (validated numerically in mathcheck.py):
 1. silu is smooth: fit it on the spline basis (gamma, max err ~2e-5) and
    fold into the coefficients -> the silu/base matmul plane disappears.
 2. B-splines form a partition of unity (sum_s B_s = 1): the constant
    direction of coefficient space becomes a per-output bias -> plane 7
    disappears. The bias enters the PSUM accumulation as the first matmul
    of each group (ones-plane x bias-row, residual-corrected bf16 rows).
 Net: 9 matmul planes -> 7 (-22% PE work). Planes+weights bf16 (rel err
 ~2.8e-3 << 2e-2 tolerance), halving W DMA and SBUF footprint.

Schedule per core (data-parallel over the 8192 tokens, 1024/core):
 - 4 token supertiles of 256. Per supertile one matmul pass accumulates the
   FULL 2048 outputs in all 8 PSUM banks, so each basis plane is consumed by
   8 matmuls (1.7us) while producing it costs ~1us -> the PE never starves
   and HAM stays warm.
 - Basis planes are built per d-chunk on DVE/Pool/ACT: one-hot cell masks
   (uint8) + blending polys, combined with copy_predicated layers (no adds).
 - W streams as 1MB batched DMAs on the sync HWDGE ring (11-tile prefetch);
   x tiles ride the ACT ring; outputs drain via gpsimd SWDGE; evacuation is
   a plain ScalarE copy (bias already accumulated in PSUM).
"""

import sys

sys.path.insert(0, '/opt/trn_rl_repo')

import numpy as np

import concourse.bass as bass  # noqa: F401  (bass must import before mybir use)
import concourse.mybir as mybir
from concourse import bacc
from concourse.tile import TileContext
from concourse.bass_utils import run_bass_kernel_spmd

F32 = mybir.dt.float32
BF16 = mybir.dt.bfloat16
U8 = mybir.dt.uint8
NP_BF16 = mybir.dt.np(BF16)
ALU = mybir.AluOpType
ACTF = mybir.ActivationFunctionType

D = 1024            # input dim
O = 2048            # output dim
NTOK = 8192         # flattened tokens
NCORES = 8
TPC = NTOK // NCORES  # tokens per core = 1024
TS = 256            # token supertile
NTS = TPC // TS     # supertiles per core = 4
NDC = D // 128      # d chunks = 8
NPL = 7             # spline planes after constant folding
NKI = NDC * NPL     # 56 contraction steps of K=128
NGRP = NKI // 8     # 7 groups of 8 ki per W DMA
OC = 512            # output chunk (one PSUM bank)
NOP = O // (2 * OC)  # oc-pairs = 2
MAGIC = 8388608.0   # 2^23 float32 round-to-nearest trick

_CACHE = {}
TRACE = False
LAST_EXEC_NS = None


def _basis(nc, tmp, bpp, planes, xtile, dc, scale, bias):
    """Emit basis ops for one d-chunk: fills planes[dc*NPL + s] (bf16) for
    s=0..6 with the un-normalized (6x) B-spline values; 1/6 is folded into
    the weights on the host."""
    TSl = xtile.shape[1]

    def ftile(tag):
        return tmp.tile([128, TSl], F32, tag=tag, name=tag)

    def btile(tag):
        return tmp.tile([128, TSl], BF16, tag=tag, name=tag)

    # scratch tiles s0..s3 (f32) are reused aggressively (hand-checked)
    s0 = ftile("s0")   # t, then u2
    s1 = ftile("s1")   # r
    s2 = ftile("s2")   # u (f32)
    s3 = ftile("s3")   # m, then u3 (f32)
    s4 = ftile("s4")   # b1t scratch
    # engine split per measured costs: DVE bf16 327ns / f32 594; ACT 612;
    # Pool 1-input ~700-800 (avoid 2-input Pool: 1111)
    nc.scalar.activation(s0[:], xtile[:], ACTF.Copy, bias=bias, scale=scale)
    nc.gpsimd.tensor_scalar(s1[:], s0[:], MAGIC, MAGIC, ALU.add, ALU.subtract)
    nc.vector.tensor_tensor(s2[:], s0[:], s1[:], ALU.subtract)   # u0 = t - r
    nc.gpsimd.tensor_scalar(s3[:], s2[:], 0.0, None, ALU.is_lt)  # m
    cell = btile("cell")
    nc.vector.tensor_tensor(cell[:], s1[:], s3[:], ALU.subtract)  # cell 0..4
    nc.vector.tensor_tensor(s2[:], s2[:], s3[:], ALU.add)        # u in [0,1)
    oh = []
    for c in range(5):
        ohc = tmp.tile([128, TSl], U8, tag=f"oh{c}", name=f"oh{c}")
        eng = nc.gpsimd if c < 2 else nc.vector
        eng.tensor_scalar(ohc[:], cell[:], float(c), None, ALU.is_equal)
        oh.append(ohc)
    nc.scalar.activation(s0[:], s2[:], ACTF.Square)              # u2 f32
    nc.vector.tensor_tensor(s3[:], s0[:], s2[:], ALU.mult)       # u3 f32
    en = btile("en")
    nc.scalar.activation(en[:], s2[:], ACTF.Copy, bias=1.0, scale=-1.0)  # 1-u
    en2 = btile("en2")
    nc.vector.tensor_tensor(en2[:], en[:], en[:], ALU.mult)      # (1-u)^2
    # 6*B blending polys: b0=(1-u)^3, b1=3u^3-6u^2+4, b2=-3u^3+3u^2+3u+1, b3=u^3
    b0 = btile("b0")
    nc.vector.tensor_tensor(b0[:], en2[:], en[:], ALU.mult)
    w1 = btile("w1")
    nc.gpsimd.tensor_scalar(w1[:], s0[:], -6.0, 4.0, ALU.mult, ALU.add)  # 4-6u^2
    b1 = btile("b1")
    nc.vector.scalar_tensor_tensor(b1[:], s3[:], 3.0, w1[:], ALU.mult,
                                   ALU.add)                      # 3u^3+4-6u^2
    p2 = btile("p2")
    nc.vector.scalar_tensor_tensor(p2[:], s3[:], -1.0, s0[:], ALU.mult,
                                   ALU.add)                      # u2-u3
    q2 = btile("q2")
    nc.scalar.activation(q2[:], s2[:], ACTF.Copy, bias=1.0, scale=3.0)   # 3u+1
    b2 = btile("b2")
    nc.vector.scalar_tensor_tensor(b2[:], p2[:], 3.0, q2[:], ALU.mult,
                                   ALU.add)
    b3 = btile("b3")
    nc.scalar.activation(b3[:], s3[:], ACTF.Copy)                # u^3 -> bf16
    bs = [b0, b1, b2, b3]
    # plane s = b_{s-cell}(u) on its support, else 0. Built as one DVE mult
    # (handles the zero background) + copy_predicated layers: overwrite dst
    # with b_{s-c} wherever cell==c. 19 DVE ops total, no adds.
    for s in range(NPL):
        dst = bpp.tile([128, TSl], BF16, tag="bp", name=f"bp{dc}_{s}")
        planes[dc * NPL + s] = dst
        cs = list(range(max(0, s - 3), min(4, s) + 1))
        nc.gpsimd.memset(dst[:], 0.0)
        for c in cs:
            nc.vector.copy_predicated(dst[:], oh[c][:], bs[s - c][:])

def _build(scale: float, bias: float, loop_reps: int = 1):
    """Per-core kernel: xt (D, TPC) f32, w (128, NGRP, 8, O) bf16,
    bvec (128, O) f32 -> out (TPC, O) f32.

    t = x*scale + bias maps x into knot-index space [0, GRID).
    loop_reps>1 wraps the body in a HW loop (for delta-reps HW timing)."""
    nc = bacc.Bacc(None, target_bir_lowering=False, debug=False)
    with TileContext(nc) as tc:
        with tc.tile_pool(name="dram", bufs=1, space="DRAM") as dram:
            xt = dram.tile([D, TPC], F32, kind="ExternalInput", tag="xt")
            w = dram.tile([128, NGRP, 8, O], BF16, kind="ExternalInput", tag="w")
            wb = dram.tile([128, O], BF16, kind="ExternalInput", tag="wb")
            out = dram.tile([TPC, O], F32, kind="ExternalOutput", tag="out")
            with (
                tc.tile_pool(name="bp", bufs=104) as bpp,
                tc.tile_pool(name="xp", bufs=10) as xpp,
                tc.tile_pool(name="tmp", bufs=2) as tmp,
                tc.tile_pool(name="wp", bufs=11) as wpp,
                tc.tile_pool(name="bv", bufs=4) as bvp,
                tc.tile_pool(name="ev", bufs=4) as evp,
                tc.tile_pool(name="wu", bufs=1) as wup,
                tc.tile_pool(name="ps", bufs=8, space="PSUM") as psp,
            ):
                # PE warmup: dummy matmuls fill the pipeline-fill window so the
                # PE p-state/HAM clock is warm when real matmuls arrive.
                # wu[:, 0:128] doubles as the all-ones plane for the bias
                # matmul (bias is accumulated into PSUM as the first matmul
                # of each group: ones.T @ (bias/128) row-replicated).
                wu = wup.tile([128, OC], BF16, tag="wu")
                nc.vector.memset(wu[:], 1.0)
                pw = psp.tile([128, OC], F32, tag="ps", name="pswarm")
                for _ in range(20):
                    nc.tensor.matmul(pw[:], wu[:, 0:128], wu[:],
                                     start=True, stop=True)
                wbtiles = []
                for oc_i in range(4):
                    wbt = bvp.tile([128, OC], BF16, tag="bv",
                                   name=f"wb{oc_i}")
                    nc.sync.dma_start(wbt[:],
                                      wb[:, oc_i * OC:(oc_i + 1) * OC])
                    wbtiles.append(wbt)

                def body():
                    for ts_i in range(NTS):
                        tok0 = ts_i * TS
                        planes = [None] * NKI
                        xtiles = []
                        for dc in range(NDC):
                            xtile = xpp.tile([128, TS], F32, tag="x",
                                             name=f"x{dc}")
                            nc.scalar.dma_start(
                                xtile[:],
                                xt[dc * 128:(dc + 1) * 128, tok0:tok0 + TS])
                            xtiles.append(xtile)
                        for dc in range(NDC):
                            _basis(nc, tmp, bpp, planes, xtiles[dc], dc,
                                   scale, bias)
                        # single matmul pass accumulating ALL 4 oc chunks:
                        # 8 psum tiles [128, OC] (one bank each) cover the
                        # whole 256-token x 2048-out supertile output. The
                        # bias enters as the first matmul of each group
                        # (ones-plane x bias-row), planes consume at 1.7us
                        # apiece vs ~1us production -> PE never starves.
                        ps = [psp.tile([128, OC], F32, tag="ps",
                                       name=f"ps{tt}_{oc}")
                              for tt in range(TS // 128) for oc in range(4)]
                        for tt in range(TS // 128):
                            for oc in range(4):
                                nc.tensor.matmul(ps[tt * 4 + oc][:],
                                                 wu[:, 0:128], wbtiles[oc][:],
                                                 start=True, stop=False)
                        for g in range(NGRP):
                            wts = []
                            for oc in range(4):
                                wt = wpp.tile([128, 8, OC], BF16, tag="w",
                                              name=f"w{oc}")
                                nc.sync.dma_start(
                                    wt[:],
                                    w[:, g, :, oc * OC:(oc + 1) * OC])
                                wts.append(wt)
                            for k in range(8):
                                ki = g * 8 + k
                                for tt in range(TS // 128):
                                    for oc in range(4):
                                        nc.tensor.matmul(
                                            ps[tt * 4 + oc][:],
                                            planes[ki][:, tt * 128:
                                                       (tt + 1) * 128],
                                            wts[oc][:, k, :],
                                            start=False,
                                            stop=(ki == NKI - 1))
                        for tt in range(TS // 128):
                            for oc in range(4):
                                ev = evp.tile([128, OC], F32, tag="ev")
                                nc.scalar.copy(ev[:], ps[tt * 4 + oc][:])
                                nc.gpsimd.dma_start(
                                    out[tok0 + tt * 128:
                                        tok0 + (tt + 1) * 128,
                                        oc * OC:(oc + 1) * OC],
                                    ev[:])

                if loop_reps > 1:
                    ET = mybir.EngineType
                    with tc.For_i(0, loop_reps, 1,
                                  hint_engines=(ET.PE, ET.DVE, ET.Pool,
                                                ET.Activation, ET.SP)):
                        body()
                elif loop_reps < 0:
                    for _ in range(-loop_reps):
                        body()
                else:
                    body()
    nc.compile()
    return nc, xt.name, w.name, wb.name, out.name


def _b_splines_np(x, grid, k):
    """Cox-de Boor in numpy (float64). x: (N,), grid: (M,) -> (N, G+k)."""
    x = x[:, None]
    g = grid[None, :]
    B = ((x >= g[:, :-1]) & (x < g[:, 1:])).astype(np.float64)
    for p in range(1, k + 1):
        left = (x - g[:, :-(p + 1)]) / (g[:, p:-1] - g[:, :-(p + 1)])
        right = (g[:, p + 1:] - x) / (g[:, p + 1:] - g[:, 1:-p])
        B = left * B[:, :-1] + right * B[:, 1:]
    return B


def _pack_host(grid, coef, scale_base, scale_sp):
    """Fold silu + constant direction into the weights; pack for the device.

    Returns (scale, bias, W[128, NGRP, 8, O] bf16, bvec[128, O] f32)."""
    g0 = np.asarray(grid[0], np.float64)          # (G+2K+1,) uniform knots
    h = float(g0[1] - g0[0])
    scale = 1.0 / h
    bias = -float(g0[3]) / h                      # t = (x - knot_K)/h

    # gamma: silu fitted on the 8 B-spline basis functions
    xs = np.linspace(float(g0[3]), float(g0[-4]) - 1e-6, 4001)
    Bs = _b_splines_np(xs, g0, 3)                 # (4001, 8)
    silu = xs / (1.0 + np.exp(-xs))
    gamma = np.linalg.lstsq(Bs, silu, rcond=None)[0]    # (8,)

    gam32 = gamma.astype(np.float32)
    C = (np.asarray(coef, np.float32) * np.asarray(scale_sp, np.float32)[:, :, None]
         + np.asarray(scale_base, np.float32)[:, :, None] * gam32[None, None, :])
    C7 = C[:, :, 7]
    bias_o = C7.sum(axis=0, dtype=np.float64)     # (O,)
    Cp = (C[:, :, :7] - C7[:, :, None]) * np.float32(1.0 / 6.0)

    W = np.empty((128, NGRP, 8, O), NP_BF16)
    for ki in range(NKI):
        dc, pl = divmod(ki, NPL)
        g_, k_ = divmod(ki, 8)
        W[:, g_, k_, :] = Cp[dc * 128:(dc + 1) * 128, :, pl].astype(NP_BF16)
    # ones-plane bias weights: rows sum to bias_o. A plain bf16(bias/128) row
    # replicated 128x quantizes coherently (2% of out std!) — correct the
    # last row with the bf16 residual instead.
    wbias = np.broadcast_to((bias_o / 128.0).astype(NP_BF16), (128, O)).copy()
    wbias[127] = (bias_o
                  - wbias[:127].astype(np.float64).sum(axis=0)).astype(NP_BF16)
    return scale, bias, W, wbias


def kernel(x, grid, coef, scale_base, scale_sp):
    assert x.shape == (4, 2048, D) and x.dtype == np.float32
    scale, bias, W, bvec = _pack_host(grid, coef, scale_base, scale_sp)
    key = (round(scale, 9), round(bias, 9))
    if key not in _CACHE:
        _CACHE[key] = _build(scale, bias)
    nc, xt_name, w_name, bv_name, out_name = _CACHE[key]

    xT = np.ascontiguousarray(x.reshape(NTOK, D).T)  # (D, NTOK)
    in_maps = []
    for c in range(NCORES):
        in_maps.append({
            xt_name: np.ascontiguousarray(xT[:, c * TPC:(c + 1) * TPC]),
            w_name: W,
            bv_name: bvec,
        })
    res = run_bass_kernel_spmd(nc, in_maps, core_ids=list(range(NCORES)),
                               trace=TRACE)
    global LAST_EXEC_NS
    LAST_EXEC_NS = res.exec_time_ns
    out = np.concatenate([res.results[c][out_name] for c in range(NCORES)],
                         axis=0)
    return out.reshape(4, 2048, O)


def _pjrt_exec(nc, in_maps):
    """Build a cached PJRT executable (no donation) + device-resident inputs.
    Returns a zero-arg callable that runs the kernel once on all 8 cores."""
    import jax
    from jax.sharding import Mesh, PartitionSpec
    from jax.experimental.shard_map import shard_map
    import concourse.mybir as _mb
    from concourse.bass2jax import (_bass_exec_p, partition_id_tensor,
                                    install_neuronx_cc_hook)
    install_neuronx_cc_hook()
    partition_name = (nc.partition_id_tensor.name
                      if nc.partition_id_tensor else None)
    in_names, out_names, out_avals, zero_outs = [], [], [], []
    for alloc in nc.m.functions[0].allocations:
        if not isinstance(alloc, _mb.MemoryLocationSet):
            continue
        name = alloc.memorylocations[0].name
        if alloc.kind == "ExternalInput":
            if name != partition_name:
                in_names.append(name)
        elif alloc.kind == "ExternalOutput":
            out_names.append(name)
            shape = tuple(alloc.tensor_shape)
            dtype = _mb.dt.np(alloc.dtype)
            out_avals.append(jax.core.ShapedArray(shape, dtype))
            zero_outs.append(np.zeros(shape, dtype))
    n_params = len(in_names)
    all_names = list(in_names) + out_names
    if partition_name is not None:
        all_names.append(partition_name)

    def _body(*args):
        operands = list(args)
        if partition_name is not None:
            operands.append(partition_id_tensor())
        outs = _bass_exec_p.bind(
            *operands, out_avals=tuple(out_avals), in_names=tuple(all_names),
            out_names=tuple(out_names), lowering_input_output_aliases=(),
            sim_require_finite=True, sim_require_nnan=True, nc=nc)
        return tuple(outs)

    n_cores = len(in_maps)
    devices = jax.devices()[:n_cores]
    mesh = Mesh(np.asarray(devices), ("core",))
    nz = len(zero_outs)
    in_specs = (PartitionSpec("core"),) * (n_params + nz)
    out_specs = (PartitionSpec("core"),) * len(out_names)
    fn = jax.jit(shard_map(_body, mesh=mesh, in_specs=in_specs,
                           out_specs=out_specs, check_rep=False),
                 keep_unused=True)
    concat_in = [np.concatenate([np.asarray(in_maps[c][nm])
                                 for c in range(n_cores)], axis=0)
                 for nm in in_names]
    concat_z = [np.zeros((n_cores * z.shape[0], *z.shape[1:]), z.dtype)
                for z in zero_outs]
    dev_args = [jax.device_put(a) for a in concat_in + concat_z]
    _ = jax.block_until_ready(fn(*dev_args))  # compile+warm

    def run():
        return jax.block_until_ready(fn(*dev_args))
    return run


def hw_time_ns(x, grid, coef, scale_base, scale_sp, r1=1, r2=101, iters=10):
    """Device-resident delta-reps timing.

    Inputs live on device and the PJRT executable is cached, so per-call
    wall = dispatch + reps * T_body. The dispatch constant is noisy
    (~85-120 ms over the axon tunnel), so r2 is large enough that the body
    signal (~reps*0.5ms) dominates; T_body comes from the median delta."""
    import time as _time
    scale, bias, W, wbias = _pack_host(grid, coef, scale_base, scale_sp)
    xT = np.ascontiguousarray(x.reshape(NTOK, D).T)
    walls = {}
    for reps in (r1, r2):
        key = (round(scale, 9), round(bias, 9), reps)
        if key not in _CACHE:
            _CACHE[key] = _build(scale, bias, loop_reps=reps)
        nc = _CACHE[key][0]
        xt_name, w_name, wb_name = _CACHE[key][1], _CACHE[key][2], _CACHE[key][3]
        in_maps = [{xt_name: np.ascontiguousarray(
                        xT[:, c * TPC:(c + 1) * TPC]),
                    w_name: W, wb_name: wbias} for c in range(NCORES)]
        run = _pjrt_exec(nc, in_maps)
        ts = []
        for _ in range(iters):
            t0 = _time.time()
            run()
            ts.append(_time.time() - t0)
        ts.sort()
        walls[reps] = ts[len(ts) // 2]
        print(f"  reps={reps}: median wall {walls[reps]*1e3:.1f} ms  "
              f"(min {ts[0]*1e3:.1f}, max {ts[-1]*1e3:.1f})")
    return (walls[r2] - walls[r1]) / (r2 - r1) * 1e9


# revision 7
# speedup vs baseline: 1.5479x; 1.5365x over previous
"""KAN projection kernel for 8x Trainium2 NeuronCores — v2.

Math: out = silu(x) @ scale_base + einsum('ndg,dog->no', B(x), coef*scale_sp)
with cubic B-splines (GRID=5, K=3 -> 8 basis functions) on a uniform grid
over [-1,1].

Reformulation (validated numerically in mathcheck.py):
 1. silu is smooth: fit it on the spline basis (gamma, max err ~2e-5) and
    fold into the coefficients -> the silu/base matmul plane disappears.
 2. B-splines form a partition of unity (sum_s B_s = 1): the constant
    direction of coefficient space becomes a per-output bias -> plane 7
    disappears. The bias enters the PSUM accumulation as the first matmul
    of each group (ones-plane x bias-row, residual-corrected bf16 rows).
 Net: 9 matmul planes -> 7 (-22% PE work). Planes+weights bf16 (rel err
 ~2.8e-3 << 2e-2 tolerance), halving W DMA and SBUF footprint.

Schedule per core (data-parallel over the 8192 tokens, 1024/core):
 - 4 token supertiles of 256. Per supertile one matmul pass accumulates the
   FULL 2048 outputs in all 8 PSUM banks, so each basis plane is consumed by
   8 matmuls (1.7us) while producing it costs ~1us -> the PE never starves
   and HAM stays warm.
 - Basis planes are built per d-chunk on DVE/Pool/ACT: one-hot cell masks
   (uint8) + blending polys, combined with copy_predicated layers (no adds).
 - W streams as 1MB batched DMAs on the sync HWDGE ring (11-tile prefetch);
   x tiles ride the ACT ring; outputs drain via gpsimd SWDGE; evacuation is
   a plain ScalarE copy (bias already accumulated in PSUM).
"""

import sys

sys.path.insert(0, '/opt/trn_rl_repo')

import numpy as np

import concourse.bass as bass  # noqa: F401  (bass must import before mybir use)
import concourse.mybir as mybir
from concourse import bacc
from concourse.tile import TileContext
from concourse.bass_utils import run_bass_kernel_spmd

F32 = mybir.dt.float32
BF16 = mybir.dt.bfloat16
U8 = mybir.dt.uint8
NP_BF16 = mybir.dt.np(BF16)
ALU = mybir.AluOpType
ACTF = mybir.ActivationFunctionType

D = 1024            # input dim
O = 2048            # output dim
NTOK = 8192         # flattened tokens
NCORES = 8
TPC = NTOK // NCORES  # tokens per core = 1024
TS = 256            # token supertile
NTS = TPC // TS     # supertiles per core = 4
NDC = D // 128      # d chunks = 8
NPL = 7             # spline planes after constant folding
NKI = NDC * NPL     # 56 contraction steps of K=128
NGRP = NKI // 8     # 7 groups of 8 ki per W DMA
OC = 512            # output chunk (one PSUM bank)
NOP = O // (2 * OC)  # oc-pairs = 2
MAGIC = 8388608.0   # 2^23 float32 round-to-nearest trick

_CACHE = {}
TRACE = False
LAST_EXEC_NS = None


def _basis(nc, tmp, bpp, planes, xtile, dc, scale, bias):
    """Emit basis ops for one d-chunk: fills planes[dc*NPL + s] (bf16) for
    s=0..6 with the un-normalized (6x) B-spline values; 1/6 is folded into
    the weights on the host."""
    TSl = xtile.shape[1]

    def ftile(tag):
        return tmp.tile([128, TSl], F32, tag=tag, name=tag)

    def btile(tag):
        return tmp.tile([128, TSl], BF16, tag=tag, name=tag)

    # scratch tiles s0..s3 (f32) are reused aggressively (hand-checked)
    s0 = ftile("s0")   # t, then u2
    s1 = ftile("s1")   # r
    s2 = ftile("s2")   # u (f32)
    s3 = ftile("s3")   # m, then u3 (f32)
    s4 = ftile("s4")   # b1t scratch
    # engine split per measured costs: DVE bf16 327ns / f32 594; ACT 612;
    # Pool 1-input ~700-800 (avoid 2-input Pool: 1111)
    nc.scalar.activation(s0[:], xtile[:], ACTF.Copy, bias=bias, scale=scale)
    nc.gpsimd.tensor_scalar(s1[:], s0[:], MAGIC, MAGIC, ALU.add, ALU.subtract)
    nc.vector.tensor_tensor(s2[:], s0[:], s1[:], ALU.subtract)   # u0 = t - r
    nc.gpsimd.tensor_scalar(s3[:], s2[:], 0.0, None, ALU.is_lt)  # m
    cell = btile("cell")
    nc.vector.tensor_tensor(cell[:], s1[:], s3[:], ALU.subtract)  # cell 0..4
    nc.vector.tensor_tensor(s2[:], s2[:], s3[:], ALU.add)        # u in [0,1)
    oh = []
    for c in range(5):
        ohc = tmp.tile([128, TSl], U8, tag=f"oh{c}", name=f"oh{c}")
        eng = nc.gpsimd if c < 2 else nc.vector
        eng.tensor_scalar(ohc[:], cell[:], float(c), None, ALU.is_equal)
        oh.append(ohc)
    nc.scalar.activation(s0[:], s2[:], ACTF.Square)              # u2 f32
    nc.vector.tensor_tensor(s3[:], s0[:], s2[:], ALU.mult)       # u3 f32
    en = btile("en")
    nc.scalar.activation(en[:], s2[:], ACTF.Copy, bias=1.0, scale=-1.0)  # 1-u
    en2 = btile("en2")
    nc.vector.tensor_tensor(en2[:], en[:], en[:], ALU.mult)      # (1-u)^2
    # 6*B blending polys: b0=(1-u)^3, b1=3u^3-6u^2+4, b2=-3u^3+3u^2+3u+1, b3=u^3
    b0 = btile("b0")
    nc.vector.tensor_tensor(b0[:], en2[:], en[:], ALU.mult)
    w1 = btile("w1")
    nc.gpsimd.tensor_scalar(w1[:], s0[:], -6.0, 4.0, ALU.mult, ALU.add)  # 4-6u^2
    b1 = btile("b1")
    nc.vector.scalar_tensor_tensor(b1[:], s3[:], 3.0, w1[:], ALU.mult,
                                   ALU.add)                      # 3u^3+4-6u^2
    p2 = btile("p2")
    nc.vector.scalar_tensor_tensor(p2[:], s3[:], -1.0, s0[:], ALU.mult,
                                   ALU.add)                      # u2-u3
    q2 = btile("q2")
    nc.scalar.activation(q2[:], s2[:], ACTF.Copy, bias=1.0, scale=3.0)   # 3u+1
    b2 = btile("b2")
    nc.vector.scalar_tensor_tensor(b2[:], p2[:], 3.0, q2[:], ALU.mult,
                                   ALU.add)
    b3 = btile("b3")
    nc.scalar.activation(b3[:], s3[:], ACTF.Copy)                # u^3 -> bf16
    bs = [b0, b1, b2, b3]
    # plane s = b_{s-cell}(u) on its support, else 0. Built as one DVE mult
    # (handles the zero background) + copy_predicated layers: overwrite dst
    # with b_{s-c} wherever cell==c. 19 DVE ops total, no adds.
    for s in range(NPL):
        dst = bpp.tile([128, TSl], BF16, tag="bp", name=f"bp{dc}_{s}")
        planes[dc * NPL + s] = dst
        cs = list(range(max(0, s - 3), min(4, s) + 1))
        nc.gpsimd.memset(dst[:], 0.0)
        for c in cs:
            nc.vector.copy_predicated(dst[:], oh[c][:], bs[s - c][:])

def _build(scale: float, bias: float, loop_reps: int = 1):
    """Per-core kernel: xt (D, TPC) f32, w (128, NGRP, 8, O) bf16,
    bvec (128, O) f32 -> out (TPC, O) f32.

    t = x*scale + bias maps x into knot-index space [0, GRID).
    loop_reps>1 wraps the body in a HW loop (for delta-reps HW timing)."""
    nc = bacc.Bacc(None, target_bir_lowering=False, debug=False)
    with TileContext(nc) as tc:
        with tc.tile_pool(name="dram", bufs=1, space="DRAM") as dram:
            xt = dram.tile([D, TPC], F32, kind="ExternalInput", tag="xt")
            w = dram.tile([128, NGRP, 8, O], BF16, kind="ExternalInput", tag="w")
            wb = dram.tile([128, O], BF16, kind="ExternalInput", tag="wb")
            out = dram.tile([TPC, O], F32, kind="ExternalOutput", tag="out")
            with (
                tc.tile_pool(name="bp", bufs=104) as bpp,
                tc.tile_pool(name="xp", bufs=10) as xpp,
                tc.tile_pool(name="tmp", bufs=2) as tmp,
                tc.tile_pool(name="wp", bufs=11) as wpp,
                tc.tile_pool(name="bv", bufs=4) as bvp,
                tc.tile_pool(name="ev", bufs=8) as evp,
                tc.tile_pool(name="wu", bufs=1) as wup,
                tc.tile_pool(name="ps", bufs=8, space="PSUM") as psp,
            ):
                # PE warmup: dummy matmuls fill the pipeline-fill window so the
                # PE p-state/HAM clock is warm when real matmuls arrive.
                # wu[:, 0:128] doubles as the all-ones plane for the bias
                # matmul (bias is accumulated into PSUM as the first matmul
                # of each group: ones.T @ (bias/128) row-replicated).
                wu = wup.tile([128, OC], BF16, tag="wu")
                nc.vector.memset(wu[:], 1.0)
                pw = psp.tile([128, OC], F32, tag="ps", name="pswarm")
                for _ in range(20):
                    nc.tensor.matmul(pw[:], wu[:, 0:128], wu[:],
                                     start=True, stop=True)
                wbtiles = []
                for oc_i in range(4):
                    wbt = bvp.tile([128, OC], BF16, tag="bv",
                                   name=f"wb{oc_i}")
                    nc.sync.dma_start(wbt[:],
                                      wb[:, oc_i * OC:(oc_i + 1) * OC])
                    wbtiles.append(wbt)

                def body():
                    for ts_i in range(NTS):
                        tok0 = ts_i * TS
                        planes = [None] * NKI
                        xtiles = []
                        for dc in range(NDC):
                            xtile = xpp.tile([128, TS], F32, tag="x",
                                             name=f"x{dc}")
                            nc.scalar.dma_start(
                                xtile[:],
                                xt[dc * 128:(dc + 1) * 128, tok0:tok0 + TS])
                            xtiles.append(xtile)
                        for dc in range(NDC):
                            _basis(nc, tmp, bpp, planes, xtiles[dc], dc,
                                   scale, bias)
                        # single matmul pass accumulating ALL 4 oc chunks:
                        # 8 psum tiles [128, OC] (one bank each) cover the
                        # whole 256-token x 2048-out supertile output. The
                        # bias enters as the first matmul of each group
                        # (ones-plane x bias-row), planes consume at 1.7us
                        # apiece vs ~1us production -> PE never starves.
                        ps = [psp.tile([128, OC], F32, tag="ps",
                                       name=f"ps{tt}_{oc}")
                              for tt in range(TS // 128) for oc in range(4)]
                        for tt in range(TS // 128):
                            for oc in range(4):
                                nc.tensor.matmul(ps[tt * 4 + oc][:],
                                                 wu[:, 0:128], wbtiles[oc][:],
                                                 start=True, stop=False)
                        for g in range(NGRP):
                            wts = []
                            for oc in range(4):
                                wt = wpp.tile([128, 8, OC], BF16, tag="w",
                                              name=f"w{oc}")
                                nc.sync.dma_start(
                                    wt[:],
                                    w[:, g, :, oc * OC:(oc + 1) * OC])
                                wts.append(wt)
                            for k in range(8):
                                ki = g * 8 + k
                                for tt in range(TS // 128):
                                    for oc in range(4):
                                        nc.tensor.matmul(
                                            ps[tt * 4 + oc][:],
                                            planes[ki][:, tt * 128:
                                                       (tt + 1) * 128],
                                            wts[oc][:, k, :],
                                            start=False,
                                            stop=(ki == NKI - 1))
                        for tt in range(TS // 128):
                            for oc in range(4):
                                ev = evp.tile([128, OC], F32, tag="ev")
                                # drain banks on two engines in parallel
                                if oc % 2 == 0:
                                    nc.scalar.copy(ev[:], ps[tt * 4 + oc][:])
                                else:
                                    nc.vector.tensor_copy(
                                        ev[:], ps[tt * 4 + oc][:])
                                nc.gpsimd.dma_start(
                                    out[tok0 + tt * 128:
                                        tok0 + (tt + 1) * 128,
                                        oc * OC:(oc + 1) * OC],
                                    ev[:])

                if loop_reps > 1:
                    ET = mybir.EngineType
                    with tc.For_i(0, loop_reps, 1,
                                  hint_engines=(ET.PE, ET.DVE, ET.Pool,
                                                ET.Activation, ET.SP)):
                        body()
                elif loop_reps < 0:
                    for _ in range(-loop_reps):
                        body()
                else:
                    body()
    nc.compile()
    return nc, xt.name, w.name, wb.name, out.name


def _b_splines_np(x, grid, k):
    """Cox-de Boor in numpy (float64). x: (N,), grid: (M,) -> (N, G+k)."""
    x = x[:, None]
    g = grid[None, :]
    B = ((x >= g[:, :-1]) & (x < g[:, 1:])).astype(np.float64)
    for p in range(1, k + 1):
        left = (x - g[:, :-(p + 1)]) / (g[:, p:-1] - g[:, :-(p + 1)])
        right = (g[:, p + 1:] - x) / (g[:, p + 1:] - g[:, 1:-p])
        B = left * B[:, :-1] + right * B[:, 1:]
    return B


def _pack_host(grid, coef, scale_base, scale_sp):
    """Fold silu + constant direction into the weights; pack for the device.

    Returns (scale, bias, W[128, NGRP, 8, O] bf16, bvec[128, O] f32)."""
    g0 = np.asarray(grid[0], np.float64)          # (G+2K+1,) uniform knots
    h = float(g0[1] - g0[0])
    scale = 1.0 / h
    bias = -float(g0[3]) / h                      # t = (x - knot_K)/h

    # gamma: silu fitted on the 8 B-spline basis functions
    xs = np.linspace(float(g0[3]), float(g0[-4]) - 1e-6, 4001)
    Bs = _b_splines_np(xs, g0, 3)                 # (4001, 8)
    silu = xs / (1.0 + np.exp(-xs))
    gamma = np.linalg.lstsq(Bs, silu, rcond=None)[0]    # (8,)

    gam32 = gamma.astype(np.float32)
    C = (np.asarray(coef, np.float32) * np.asarray(scale_sp, np.float32)[:, :, None]
         + np.asarray(scale_base, np.float32)[:, :, None] * gam32[None, None, :])
    C7 = C[:, :, 7]
    bias_o = C7.sum(axis=0, dtype=np.float64)     # (O,)
    Cp = (C[:, :, :7] - C7[:, :, None]) * np.float32(1.0 / 6.0)

    W = np.empty((128, NGRP, 8, O), NP_BF16)
    for ki in range(NKI):
        dc, pl = divmod(ki, NPL)
        g_, k_ = divmod(ki, 8)
        W[:, g_, k_, :] = Cp[dc * 128:(dc + 1) * 128, :, pl].astype(NP_BF16)
    # ones-plane bias weights: rows sum to bias_o. A plain bf16(bias/128) row
    # replicated 128x quantizes coherently (2% of out std!) — correct the
    # last row with the bf16 residual instead.
    wbias = np.broadcast_to((bias_o / 128.0).astype(NP_BF16), (128, O)).copy()
    wbias[127] = (bias_o
                  - wbias[:127].astype(np.float64).sum(axis=0)).astype(NP_BF16)
    return scale, bias, W, wbias


def kernel(x, grid, coef, scale_base, scale_sp):
    assert x.shape == (4, 2048, D) and x.dtype == np.float32
    scale, bias, W, bvec = _pack_host(grid, coef, scale_base, scale_sp)
    key = (round(scale, 9), round(bias, 9))
    if key not in _CACHE:
        _CACHE[key] = _build(scale, bias)
    nc, xt_name, w_name, bv_name, out_name = _CACHE[key]

    xT = np.ascontiguousarray(x.reshape(NTOK, D).T)  # (D, NTOK)
    in_maps = []
    for c in range(NCORES):
        in_maps.append({
            xt_name: np.ascontiguousarray(xT[:, c * TPC:(c + 1) * TPC]),
            w_name: W,
            bv_name: bvec,
        })
    res = run_bass_kernel_spmd(nc, in_maps, core_ids=list(range(NCORES)),
                               trace=TRACE)
    global LAST_EXEC_NS
    LAST_EXEC_NS = res.exec_time_ns
    out = np.concatenate([res.results[c][out_name] for c in range(NCORES)],
                         axis=0)
    return out.reshape(4, 2048, O)


def _pjrt_exec(nc, in_maps):
    """Build a cached PJRT executable (no donation) + device-resident inputs.
    Returns a zero-arg callable that runs the kernel once on all 8 cores."""
    import jax
    from jax.sharding import Mesh, PartitionSpec
    from jax.experimental.shard_map import shard_map
    import concourse.mybir as _mb
    from concourse.bass2jax import (_bass_exec_p, partition_id_tensor,
                                    install_neuronx_cc_hook)
    install_neuronx_cc_hook()
    partition_name = (nc.partition_id_tensor.name
                      if nc.partition_id_tensor else None)
    in_names, out_names, out_avals, zero_outs = [], [], [], []
    for alloc in nc.m.functions[0].allocations:
        if not isinstance(alloc, _mb.MemoryLocationSet):
            continue
        name = alloc.memorylocations[0].name
        if alloc.kind == "ExternalInput":
            if name != partition_name:
                in_names.append(name)
        elif alloc.kind == "ExternalOutput":
            out_names.append(name)
            shape = tuple(alloc.tensor_shape)
            dtype = _mb.dt.np(alloc.dtype)
            out_avals.append(jax.core.ShapedArray(shape, dtype))
            zero_outs.append(np.zeros(shape, dtype))
    n_params = len(in_names)
    all_names = list(in_names) + out_names
    if partition_name is not None:
        all_names.append(partition_name)

    def _body(*args):
        operands = list(args)
        if partition_name is not None:
            operands.append(partition_id_tensor())
        outs = _bass_exec_p.bind(
            *operands, out_avals=tuple(out_avals), in_names=tuple(all_names),
            out_names=tuple(out_names), lowering_input_output_aliases=(),
            sim_require_finite=True, sim_require_nnan=True, nc=nc)
        return tuple(outs)

    n_cores = len(in_maps)
    devices = jax.devices()[:n_cores]
    mesh = Mesh(np.asarray(devices), ("core",))
    nz = len(zero_outs)
    in_specs = (PartitionSpec("core"),) * (n_params + nz)
    out_specs = (PartitionSpec("core"),) * len(out_names)
    fn = jax.jit(shard_map(_body, mesh=mesh, in_specs=in_specs,
                           out_specs=out_specs, check_rep=False),
                 keep_unused=True)
    concat_in = [np.concatenate([np.asarray(in_maps[c][nm])
                                 for c in range(n_cores)], axis=0)
                 for nm in in_names]
    concat_z = [np.zeros((n_cores * z.shape[0], *z.shape[1:]), z.dtype)
                for z in zero_outs]
    dev_args = [jax.device_put(a) for a in concat_in + concat_z]
    _ = jax.block_until_ready(fn(*dev_args))  # compile+warm

    def run():
        return jax.block_until_ready(fn(*dev_args))
    return run


def hw_time_ns(x, grid, coef, scale_base, scale_sp, r1=1, r2=101, iters=10):
    """Device-resident delta-reps timing.

    Inputs live on device and the PJRT executable is cached, so per-call
    wall = dispatch + reps * T_body. The dispatch constant is noisy
    (~85-120 ms over the axon tunnel), so r2 is large enough that the body
    signal (~reps*0.5ms) dominates; T_body comes from the median delta."""
    import time as _time
    scale, bias, W, wbias = _pack_host(grid, coef, scale_base, scale_sp)
    xT = np.ascontiguousarray(x.reshape(NTOK, D).T)
    walls = {}
    for reps in (r1, r2):
        key = (round(scale, 9), round(bias, 9), reps)
        if key not in _CACHE:
            _CACHE[key] = _build(scale, bias, loop_reps=reps)
        nc = _CACHE[key][0]
        xt_name, w_name, wb_name = _CACHE[key][1], _CACHE[key][2], _CACHE[key][3]
        in_maps = [{xt_name: np.ascontiguousarray(
                        xT[:, c * TPC:(c + 1) * TPC]),
                    w_name: W, wb_name: wbias} for c in range(NCORES)]
        run = _pjrt_exec(nc, in_maps)
        ts = []
        for _ in range(iters):
            t0 = _time.time()
            run()
            ts.append(_time.time() - t0)
        ts.sort()
        walls[reps] = ts[len(ts) // 2]
        print(f"  reps={reps}: median wall {walls[reps]*1e3:.1f} ms  "
              f"(min {ts[0]*1e3:.1f}, max {ts[-1]*1e3:.1f})")
    return (walls[r2] - walls[r1]) / (r2 - r1) * 1e9
